# revision 1
# baseline (speedup 1.0000x reference)
"""Fused multi-head attention block (QKV proj + RMSNorm + 2D RoPE + softmax
attention + out proj) for Trainium2, data-parallel over batch on 8 NeuronCores.

Layout strategy per core (one batch element, N=1024 tokens, D=1024, H=16, hd=64):
  - x is PE-transposed to xT [D, N] once.
  - Q,K are produced transposed ("qkT" [feat, n]) so attention scores need no
    further transposes; V is produced in natural [n, feat] layout (it is the
    stationary operand of the AV matmul), augmented with a ones column so the
    softmax denominator falls out of the same accumulation.
  - RMSNorm variance is a partition-dim reduction -> block-ones matmul on PE.
  - RoPE rotate-half runs as two DVE table-multiplies plus a PE swap-matrix
    matmul (the +/- signs and q/k_scale are folded into host-built tables).
  - rstd / softmax-denominator broadcasts across partitions use 0-stride DMA.
  - All matmuls run in float32r (tf32) which streams at bf16 rate on the PE.
Softmax skips max-subtraction: after RMSNorm ||q||<=8, ||k||<=8 so logits are
within [-64,64]*hd^-0.5 = [-8,8], safely inside fp32 exp range.
"""

import sys

sys.path.insert(0, "/opt/trn_rl_repo")

import numpy as np

_BUILT = None

B, N, D = 8, 1024, 1024
H, HD = 16, 64
P = 128
NB = 2          # free-dim blocks of 512 over n
FB = 512        # matmul free-dim block
KT = D // P     # 8 contraction chunks
NT = N // P     # 8 n-chunks
THETA = 10000.0
EPS = 1e-6


def _round_tf32(a: np.ndarray) -> np.ndarray:
    """Round fp32 to tf32 (10 explicit mantissa bits), round-to-nearest-even."""
    v = a.astype(np.float32).view(np.uint32)
    lsb = (v >> 13) & 1
    v = v + 0x0FFF + lsb
    v = v & np.uint32(0xFFFFE000)
    return v.view(np.float32)


def _rope_tables():
    side = int(np.sqrt(N))
    dq = HD // 4
    inv_freq = 1.0 / (THETA ** (np.arange(dq, dtype=np.float32) / dq))
    ang = np.arange(side, dtype=np.float32)[:, None] * inv_freq[None, :]
    row = np.broadcast_to(ang[:, None, :], (side, side, dq)).reshape(N, dq)
    col = np.broadcast_to(ang[None, :, :], (side, side, dq)).reshape(N, dq)
    angles = np.concatenate([row, col], axis=-1)  # [N, 32]
    return np.cos(angles), np.sin(angles)


def _build_tables(scale_vec: np.ndarray):
    """cosF/sinF' [128, N] for a 2-head tile (rows: head-even dims 0..63, then
    head-odd dims 0..63). cosF[i] = cos(a_{i%32}) * s[i].
    sinF'[i] = +sin(a_{i%32})*s[i] for (i%64)<32 else -sin(a_{i%32})*s[i]."""
    cos, sin = _rope_tables()  # [N, 32] each
    cosF = np.empty((P, N), np.float32)
    sinF = np.empty((P, N), np.float32)
    for i in range(P):
        d = i % HD            # dim within head
        a = d % 32            # angle index
        s = scale_vec[d]
        cosF[i] = cos[:, a] * s
        sinF[i] = (sin[:, a] * s) * (1.0 if d < 32 else -1.0)
    return cosF, sinF


def _build_program():
    import concourse.bass as bass
    import concourse.mybir as mybir
    import concourse.tile as tile
    from concourse import bacc
    from concourse.bass import ds
    from concourse import hw_specs

    # Keep every ACT function this kernel uses (ln, exp, copy, identity) in a
    # single table set so the table-load pass emits exactly one load instead
    # of thrashing between exp/sqrt/ln sets (~2.7us per switch).
    if not getattr(bacc, "_act_tables_patched", False):
        _orig_get_tables = bacc.get_activation_tables

        def _only_lnexp(arch):
            import concourse.mybir as _mb
            tabs = _orig_get_tables(arch)
            if "natural_log_exp_and_others" not in tabs:
                return tabs
            # act_func_set_id is positional, so keep every entry in place and
            # instead make natural_log_exp_and_others the only set offering
            # the functions this kernel uses.
            steer = set()
            for fname in ("Exp", "Ln", "Copy", "Identity", "Square"):
                steer.add(getattr(_mb.ActivationFunctionType, fname))
            out = {}
            for name, funcs in tabs.items():
                if name == "natural_log_exp_and_others":
                    out[name] = funcs
                else:
                    out[name] = funcs - steer
            return out

        bacc.get_activation_tables = _only_lnexp
        bacc._act_tables_patched = True

    F32R = mybir.dt.float32r
    FP32 = mybir.dt.float32
    AF = mybir.ActivationFunctionType

    nc = bacc.Bacc("TRN2", target_bir_lowering=False, debug=False, num_devices=8)

    x = nc.dram_tensor("x", [N, D], FP32, kind="ExternalInput").ap()
    wqkv = nc.dram_tensor("wqkv", [D, 3 * D], F32R, kind="ExternalInput").ap()
    wout = nc.dram_tensor("wout", [D, D], F32R, kind="ExternalInput").ap()
    bqkv_cols_d = nc.dram_tensor("bqkv_cols", [P, 2 * KT], FP32, kind="ExternalInput").ap()
    bqkv_v_d = nc.dram_tensor("bqkv_v", [1, D], F32R, kind="ExternalInput").ap()
    bout_d = nc.dram_tensor("bout_r", [1, D], F32R, kind="ExternalInput").ap()
    cosf_d = nc.dram_tensor("cosf", [P, N], FP32, kind="ExternalInput").ap()
    sinf_d = nc.dram_tensor("sinf", [P, N], FP32, kind="ExternalInput").ap()
    ident_d = nc.dram_tensor("ident", [P, P], FP32, kind="ExternalInput").ap()
    ident_r_d = nc.dram_tensor("ident_r", [P, P], F32R, kind="ExternalInput").ap()
    swap_d = nc.dram_tensor("swapm", [P, P], F32R, kind="ExternalInput").ap()
    ones2q_d = nc.dram_tensor("ones2q", [P, 2], F32R, kind="ExternalInput").ap()
    ones2k_d = nc.dram_tensor("ones2k", [P, 2], F32R, kind="ExternalInput").ap()
    ones1_d = nc.dram_tensor("ones1", [1, P], F32R, kind="ExternalInput").ap()
    vones_d = nc.dram_tensor("vones", [P, NT, H, 1], F32R, kind="ExternalInput").ap()
    out = nc.dram_tensor("out", [N, D], FP32, kind="ExternalOutput").ap()
    rstdq_d = nc.dram_tensor("rstdq_scratch", [H, N], FP32).ap()
    den_d = nc.dram_tensor("den_scratch", [H, N], FP32).ap()
    oT_d = nc.dram_tensor("oT_scratch", [D, N], FP32).ap()   # f32r bits

    with tile.TileContext(nc) as tc:
        with tc.tile_pool(name="big", bufs=1) as big, \
             tc.tile_pool(name="tab", bufs=1) as tab:
            qkT = big.tile([P, 2 * KT, N], F32R)      # tile t: heads 2t,2t+1
            vaug = big.tile([P, NT, H, HD + 1], F32R)  # V natural + ones column

            cosf = tab.tile([P, N], FP32)
            sinf = tab.tile([P, N], FP32)
            ident = tab.tile([P, P], FP32)
            ident_r = tab.tile([P, P], F32R)
            swapm = tab.tile([P, P], F32R)
            ones2q = tab.tile([P, 2], F32R)
            ones2k = tab.tile([P, 2], F32R)
            ones1 = tab.tile([1, P], F32R)
            bqkv_cols = tab.tile([P, 2 * KT], FP32)
            bqkv_v = tab.tile([1, D], F32R)
            bout_t = tab.tile([1, D], F32R)
            rstdkT = tab.tile([P, NT, H], FP32)   # 0.125/sigma_k
            eps_t = tab.tile([P, 1], FP32)
            eps64_t = tab.tile([P, 1], FP32)
            zero_t = tab.tile([P, 1], FP32)
            ln8_t = tab.tile([P, 1], FP32)

            for dst, src in [(cosf, cosf_d), (sinf, sinf_d),
                             (ident, ident_d), (ident_r, ident_r_d),
                             (swapm, swap_d), (ones2q, ones2q_d), (ones2k, ones2k_d),
                             (ones1, ones1_d),
                             (bqkv_cols, bqkv_cols_d), (bqkv_v, bqkv_v_d),
                             (bout_t, bout_d)]:
                nc.sync.dma_start(out=dst, in_=src)

            nc.sync.dma_start(out=vaug[:, :, :, HD:HD + 1], in_=vones_d)
            nc.vector.memset(eps_t, EPS)
            nc.vector.memset(eps64_t, EPS * HD)
            nc.vector.memset(zero_t, 0.0)
            nc.vector.memset(ln8_t, -2.0794415416798357)  # ln(1/8)

            with tc.tile_pool(name="xTp", bufs=1) as xTp:
                xT = xTp.tile([P, KT, N], F32R)

                # -------- Phase 1a: xT = transpose(x) --------
                with tc.tile_pool(name="xin", bufs=2) as xin, \
                     tc.tile_pool(name="psxp", bufs=3, space="PSUM") as psxp:
                    for k in range(KT):
                        xcol = xin.tile([P, NT, P], FP32, tag="xcol")
                        nc.sync.dma_start(
                            out=xcol,
                            in_=x[:, ds(k * P, P)].rearrange(
                                "(no ni) d -> ni no d", ni=P),
                        )
                        for half in range(NB):
                            pxt = psxp.tile([P, FB], FP32, tag="pxt")
                            for j in range(4):
                                nc.tensor.transpose(
                                    pxt[:, ds(j * P, P)],
                                    xcol[:, 4 * half + j, :],
                                    ident,
                                )
                            nc.vector.tensor_copy(
                                out=xT[:, k, ds(half * FB, FB)], in_=pxt)

                # -------- V projection (256-wide blocks) --------
                with tc.tile_pool(name="wv", bufs=9) as wvp, \
                     tc.tile_pool(name="psv", bufs=3, space="PSUM") as psv:
                    VB = 256
                    for fb in range(D // VB):
                        wvs = []
                        for k in range(KT):
                            wv = wvp.tile([P, VB], F32R, tag="wv")
                            nc.sync.dma_start(
                                out=wv,
                                in_=wqkv[ds(k * P, P), ds(2 * D + fb * VB, VB)])
                            wvs.append(wv)
                        for mc in range(NT):
                            pv = psv.tile([P, VB], FP32, tag="pv")
                            for k in range(KT):
                                nc.tensor.matmul(
                                    pv, xT[:, k, ds(mc * P, P)], wvs[k],
                                    start=(k == 0), stop=False)
                            nc.tensor.matmul(
                                pv, ones1, bqkv_v[:, ds(fb * VB, VB)],
                                start=False, stop=True)
                            nc.vector.tensor_copy(
                                out=vaug[:, mc, ds(fb * 4, 4), 0:HD],
                                in_=pv.rearrange("p (h d) -> p h d", h=4))

                # -------- fused per-head-pair pipeline --------
                with tc.tile_pool(name="wqk", bufs=3) as wqkp, \
                     tc.tile_pool(name="sq", bufs=4) as sqp, \
                     tc.tile_pool(name="uc", bufs=5) as ucp, \
                     tc.tile_pool(name="bcp", bufs=4) as bcp, \
                     tc.tile_pool(name="ex", bufs=4) as exp_p, \
                     tc.tile_pool(name="rcp", bufs=2) as rcp, \
                     tc.tile_pool(name="oTs", bufs=2) as oTsp, \
                     tc.tile_pool(name="psmm", bufs=6, space="PSUM") as psmm, \
                     tc.tile_pool(name="psav", bufs=2, space="PSUM") as psav:
                    def emit_proj_stats(pg):
                        for t in (pg, KT + pg):
                            # qkT projection for feature tile t
                            wcol = wqkp.tile([P, KT, P], F32R, tag="wc")
                            nc.sync.dma_start(
                                out=wcol,
                                in_=wqkv[:, ds(t * P, P)].rearrange(
                                    "(ko ki) f -> ki ko f", ki=P))
                            for nb in range(NB):
                                pm = psmm.tile([P, FB], FP32, tag="mm")
                                for k in range(KT):
                                    nc.tensor.matmul(
                                        pm, wcol[:, k, :], xT[:, k, ds(nb * FB, FB)],
                                        start=(k == 0), stop=(k == KT - 1))
                                nc.vector.tensor_scalar_add(
                                    out=qkT[:, t, ds(nb * FB, FB)], in0=pm,
                                    scalar1=bqkv_cols[:, t:t + 1])

                            # RMSNorm stats for tile t
                            for nb in range(NB):
                                sl = ds(nb * FB, FB)
                                qs = qkT[:, t, sl]
                                sq = sqp.tile([P, FB], F32R, tag="sq")
                                nc.vector.tensor_mul(
                                    out=sq, in0=qs.bitcast(FP32), in1=qs.bitcast(FP32))
                                if t < KT:
                                    pm = psmm.tile([P, FB], FP32, tag="mm")
                                    pss = pm[0:2, :]
                                    nc.tensor.matmul(pss, ones2q, sq, start=True, stop=True)
                                    sg = sqp.tile([2, FB], FP32, tag="sq")
                                    nc.scalar.activation(
                                        out=sg, in_=pss,
                                        func=AF.Ln, scale=1.0 / HD, bias=eps_t[0:2, :])
                                    sg2 = sqp.tile([2, FB], FP32, tag="sq")
                                    nc.scalar.activation(
                                        out=sg2, in_=sg,
                                        func=AF.Exp, scale=-0.5, bias=zero_t[0:2, :])
                                    nc.sync.dma_start(
                                        out=rstdq_d[2 * pg:2 * pg + 2, sl], in_=sg2)
                                else:
                                    for j in range(4):
                                        mc = nb * 4 + j
                                        pm = psmm.tile([P, FB], FP32, tag="mm")
                                        psT = pm[:, 0:2]
                                        nc.tensor.matmul(psT, sq[:, ds(j * P, P)], ones2k,
                                                         start=True, stop=True)
                                        # 0.125/sigma_k = exp(-0.5*ln(sumsq+HD*eps) + ln(1/8))
                                        lt = sqp.tile([P, 2], FP32, tag="lt")
                                        nc.scalar.activation(
                                            out=lt, in_=psT,
                                            func=AF.Ln, scale=1.0, bias=eps64_t)
                                        nc.scalar.activation(
                                            out=rstdkT[:, mc, 2 * pg:2 * pg + 2], in_=lt,
                                            func=AF.Exp, scale=-0.5, bias=zero_t)


                    def emit_rope(pg):
                        for t in (pg, KT + pg):
                            # RoPE (+ rstd_q apply for Q tiles), in place
                            for nb in range(NB):
                                sl = ds(nb * FB, FB)
                                qs32 = qkT[:, t, sl].bitcast(FP32)
                                u = ucp.tile([P, FB], F32R, tag="uc")
                                c = ucp.tile([P, FB], F32R, tag="uc")
                                nc.vector.tensor_mul(out=u, in0=qs32, in1=sinf[:, sl])
                                nc.vector.tensor_mul(out=c, in0=qs32, in1=cosf[:, sl])
                                pr = psmm.tile([P, FB], FP32, tag="mm")
                                nc.tensor.matmul(pr, swapm, u, start=True, stop=False)
                                nc.tensor.matmul(pr, ident_r, c, start=False, stop=True)
                                if t < KT:
                                    bc = bcp.tile([P, FB], FP32, tag="bc")
                                    nc.sync.dma_start(
                                        out=bc[0:HD],
                                        in_=rstdq_d[2 * pg:2 * pg + 1, sl].broadcast_to([HD, FB]))
                                    nc.sync.dma_start(
                                        out=bc[HD:P],
                                        in_=rstdq_d[2 * pg + 1:2 * pg + 2, sl].broadcast_to([HD, FB]))
                                    nc.vector.tensor_mul(
                                        out=qkT[:, t, sl], in0=pr, in1=bc)
                                else:
                                    nc.vector.tensor_copy(out=qkT[:, t, sl], in_=pr)



                    def emit_att(pg):
                        # attention for heads (2pg, 2pg+1)
                        for nb in range(NB):
                            sl = ds(nb * FB, FB)
                            av0 = psav.tile([HD + 1, FB], FP32, tag="av")
                            av1 = psav.tile([HD + 1, FB], FP32, tag="av")
                            for mc in range(NT):
                                sp0 = psmm.tile([P, FB], FP32, tag="mm")
                                sp1 = psmm.tile([P, FB], FP32, tag="mm")
                                nc.tensor.matmul(
                                    sp0, qkT[0:HD, KT + pg, ds(mc * P, P)],
                                    qkT[0:HD, pg, sl], start=True, stop=True)
                                nc.tensor.matmul(
                                    sp1, qkT[HD:P, KT + pg, ds(mc * P, P)],
                                    qkT[HD:P, pg, sl], start=True, stop=True)
                                e0 = exp_p.tile([P, FB], F32R, tag="e")
                                e1 = exp_p.tile([P, FB], F32R, tag="e")
                                nc.scalar.activation(out=e0, in_=sp0, func=AF.Exp,
                                                     scale=rstdkT[:, mc, 2 * pg:2 * pg + 1])
                                nc.scalar.activation(out=e1, in_=sp1, func=AF.Exp,
                                                     scale=rstdkT[:, mc, 2 * pg + 1:2 * pg + 2])
                                nc.tensor.matmul(av0, vaug[:, mc, 2 * pg, :], e0,
                                                 start=(mc == 0), stop=(mc == NT - 1))
                                nc.tensor.matmul(av1, vaug[:, mc, 2 * pg + 1, :], e1,
                                                 start=(mc == 0), stop=(mc == NT - 1))
                            for hh, av in ((0, av0), (1, av1)):
                                avs = oTsp.tile([HD + 1, FB], FP32, tag="avs")
                                nc.vector.tensor_copy(out=avs, in_=av)
                                rec = rcp.tile([1, FB], FP32, tag="rec")
                                nc.vector.reciprocal(out=rec, in_=avs[HD:HD + 1, :])
                                nc.sync.dma_start(
                                    out=den_d[2 * pg + hh:2 * pg + hh + 1, sl], in_=rec)
                                bcd = bcp.tile([P, FB], FP32, tag="bc")
                                nc.sync.dma_start(
                                    out=bcd[0:HD],
                                    in_=den_d[2 * pg + hh:2 * pg + hh + 1, sl].broadcast_to([HD, FB]))
                                ot = oTsp.tile([HD, FB], F32R, tag="ot")
                                nc.vector.tensor_mul(
                                    out=ot, in0=avs[0:HD, :], in1=bcd[0:HD])
                                nc.sync.dma_start(
                                    out=oT_d[ds(pg * P + hh * HD, HD), sl],
                                    in_=ot.bitcast(FP32))


                    for pg in range(KT):
                        emit_proj_stats(pg)
                        emit_rope(pg)
                        if pg >= 1:
                            emit_att(pg - 1)
                    emit_att(KT - 1)

            # -------- Phase 4: output projection (oT streamed from DRAM) ----
            with tc.tile_pool(name="wo", bufs=10) as wop, \
                 tc.tile_pool(name="oin", bufs=3) as oinp, \
                 tc.tile_pool(name="oout", bufs=3) as ooutp, \
                 tc.tile_pool(name="pso", bufs=3, space="PSUM") as pso:
                for ob in range(NB):
                    wos = []
                    for k in range(KT):
                        wo = wop.tile([P, FB], F32R, tag="wo")
                        nc.sync.dma_start(
                            out=wo, in_=wout[ds(k * P, P), ds(ob * FB, FB)])
                        wos.append(wo)
                    for nch in range(NT):
                        ocol = oinp.tile([P, KT, P], F32R, tag="oc")
                        nc.sync.dma_start(
                            out=ocol,
                            in_=oT_d[:, ds(nch * P, P)].rearrange(
                                "(ko ki) f -> ki ko f", ki=P).bitcast(F32R))
                        po = pso.tile([P, FB], FP32, tag="po")
                        for k in range(KT):
                            nc.tensor.matmul(
                                po, ocol[:, k, :], wos[k],
                                start=(k == 0), stop=False)
                        nc.tensor.matmul(po, ones1, bout_t[:, ds(ob * FB, FB)],
                                         start=False, stop=True)
                        osb = ooutp.tile([P, FB], FP32, tag="osb")
                        nc.scalar.copy(out=osb, in_=po)
                        nc.sync.dma_start(
                            out=out[ds(nch * P, P), ds(ob * FB, FB)], in_=osb)

    nc.compile()
    return nc


def _host_inputs(Wqkv, bqkv, Wout, bout, q_scale, k_scale):
    cosF, sinF = _build_tables(np.ones(HD, np.float32))

    ident = np.eye(P, dtype=np.float32)
    swapm = np.zeros((P, P), np.float32)
    for k in range(P):
        m = (k & ~63) + ((k & 63) ^ 32)
        swapm[k, m] = 1.0
    ones1 = np.ones((1, P), np.float32)

    # Fold q/k_scale into the Q/K projection columns; the RMSNorm variance of
    # the *unscaled* q is then recovered with a 1/scale^2-weighted reduction.
    qs = q_scale.astype(np.float32)
    ks = k_scale.astype(np.float32)
    W = Wqkv.astype(np.float32).copy()
    b = bqkv.astype(np.float32).copy()
    qcol = np.tile(qs, H)      # [D] scale per q feature
    kcol = np.tile(ks, H)
    W[:, 0:D] *= qcol[None, :]
    W[:, D:2 * D] *= kcol[None, :]
    b[0:D] *= qcol
    b[D:2 * D] *= kcol

    def wones(sv):
        o = np.zeros((P, 2), np.float32)
        inv2 = 1.0 / (sv * sv)
        o[0:HD, 0] = inv2
        o[HD:P, 1] = inv2
        return o

    bqkv_cols = np.ascontiguousarray(
        b[:2 * D].reshape(2 * KT, P).T).astype(np.float32)

    return {
        "wqkv": _round_tf32(W),
        "wout": _round_tf32(Wout),
        "bqkv_cols": bqkv_cols,
        "bqkv_v": _round_tf32(b[2 * D:].reshape(1, D)),
        "bout_r": _round_tf32(bout.reshape(1, D)),
        "cosf": cosF, "sinf": sinF,
        "ident": ident, "ident_r": ident, "swapm": swapm,
        "ones2q": wones(qs), "ones2k": wones(ks), "ones1": ones1,
        "vones": np.ones((P, NT, H, 1), np.float32),
    }


def _get_built():
    global _BUILT
    if _BUILT is None:
        _BUILT = _build_program()
    return _BUILT


def kernel(x, Wqkv, bqkv, Wout, bout, q_scale, k_scale, _trace=False):
    from concourse.bass_utils import run_bass_kernel_spmd

    x = np.asarray(x, dtype=np.float32)
    shared = _host_inputs(np.asarray(Wqkv, np.float32), np.asarray(bqkv, np.float32),
                          np.asarray(Wout, np.float32), np.asarray(bout, np.float32),
                          np.asarray(q_scale, np.float32), np.asarray(k_scale, np.float32))
    in_maps = [dict(shared, x=np.ascontiguousarray(x[c])) for c in range(B)]
    nc = _get_built()
    res = run_bass_kernel_spmd(nc, in_maps, core_ids=list(range(B)), trace=_trace)
    out = np.stack([res.results[c]["out"] for c in range(B)], axis=0)
    kernel.last_exec_time_ns = res.exec_time_ns
    kernel.last_results = res
    return out



# revision 11
# speedup vs baseline: 1.5391x; 1.5391x over previous
"""Fused multi-head attention block (QKV proj + RMSNorm + 2D RoPE + softmax
attention + out proj) for Trainium2, data-parallel over batch on 8 NeuronCores.

v2 layout strategy per core (one batch element, N=1024, D=1024, H=16, hd=64):
  - All PE operands are bf16 (weights host-cast; x cast on-chip); PSUM
    accumulates fp32. bf16 enables fast-weight-load and 2x DVE modes.
  - x is transposed to xT [D, N] by the DMA XBAR (dma_start_transpose), not
    the PE.
  - Q,K are produced transposed ("qkT" [feat, n]); V in natural [n, feat]
    layout augmented with a ones column so the softmax denominator falls out
    of the AV accumulation.
  - Matmuls write [128, 1024] two-bank PSUM groups so one weight load
    streams 1024 columns (halves LDWEIGHTS count).
  - RMSNorm sumsq is computed per 128-token chunk into a [128, 8, 2]
    partition-major PSUM tile -> two small ACTs (ln, exp) per feature tile.
    Q-side rstd is applied via 0-stride broadcast DMA after RoPE; K-side
    rstd (with hd^-0.5 folded in) becomes the per-partition scale of the
    softmax exp ACT.
  - RoPE rotate-half: two DVE table-multiplies, one PE swap-matrix matmul,
    and a DVE add (no identity matmul).
  - Softmax denominator reciprocals are batched into one [128, 16] DVE op
    per head pair via a DRAM-rearrange round trip; the 1/den multiply runs
    on the (otherwise idle) GpSimd engine.
  - Attention output oT stays in SBUF for the final projection.
Softmax skips max-subtraction: after RMSNorm ||q||<=8, ||k||<=8 so logits
are within [-8, 8], safely inside exp range.
"""

import sys

sys.path.insert(0, "/opt/trn_rl_repo")

import numpy as np

_BUILT = None

B, N, D = 8, 1024, 1024
H, HD = 16, 64
P = 128
NB = 2          # free-dim blocks of 512 over n
FB = 512
KT = D // P     # 8 contraction chunks
NT = N // P     # 8 n-chunks
THETA = 10000.0
EPS = 1e-6


def _rope_tables():
    side = int(np.sqrt(N))
    dq = HD // 4
    inv_freq = 1.0 / (THETA ** (np.arange(dq, dtype=np.float32) / dq))
    ang = np.arange(side, dtype=np.float32)[:, None] * inv_freq[None, :]
    row = np.broadcast_to(ang[:, None, :], (side, side, dq)).reshape(N, dq)
    col = np.broadcast_to(ang[None, :, :], (side, side, dq)).reshape(N, dq)
    angles = np.concatenate([row, col], axis=-1)  # [N, 32]
    return np.cos(angles), np.sin(angles)


def _build_tables():
    """cosF/sinF' [128, N] for a 2-head tile (rows: head-even dims 0..63,
    then head-odd dims 0..63). sinF'[i] carries the rotate-half sign."""
    cos, sin = _rope_tables()  # [N, 32] each
    cosF = np.empty((P, N), np.float32)
    sinF = np.empty((P, N), np.float32)
    for i in range(P):
        d = i % HD
        a = d % 32
        cosF[i] = cos[:, a]
        sinF[i] = sin[:, a] * (1.0 if d < 32 else -1.0)
    return cosF, sinF


def _build_program():
    import concourse.bass as bass
    import concourse.mybir as mybir
    import concourse.tile as tile
    from concourse import bacc
    from concourse.bass import ds

    # Keep every ACT function this kernel uses (ln, exp, copy) in a single
    # table set so the table-load pass emits exactly one load.
    if not getattr(bacc, "_act_tables_patched", False):
        _orig_get_tables = bacc.get_activation_tables

        def _only_lnexp(arch):
            import concourse.mybir as _mb
            tabs = _orig_get_tables(arch)
            if "natural_log_exp_and_others" not in tabs:
                return tabs
            steer = set()
            for fname in ("Exp", "Ln", "Copy", "Identity", "Square"):
                steer.add(getattr(_mb.ActivationFunctionType, fname))
            out = {}
            for name, funcs in tabs.items():
                if name == "natural_log_exp_and_others":
                    out[name] = funcs
                else:
                    out[name] = funcs - steer
            return out

        bacc.get_activation_tables = _only_lnexp
        bacc._act_tables_patched = True

    BF16 = mybir.dt.bfloat16
    FP32 = mybir.dt.float32
    AF = mybir.ActivationFunctionType

    nc = bacc.Bacc("TRN2", target_bir_lowering=False, debug=False, num_devices=8)

    x = nc.dram_tensor("x", [N, D], FP32, kind="ExternalInput").ap()
    wqkv = nc.dram_tensor("wqkv", [D, 3 * D], BF16, kind="ExternalInput").ap()
    wout = nc.dram_tensor("wout", [D, D], BF16, kind="ExternalInput").ap()
    bqkv_cols_d = nc.dram_tensor("bqkv_cols", [P, 2 * KT], FP32, kind="ExternalInput").ap()
    bv_row_d = nc.dram_tensor("bv_row", [1, D], BF16, kind="ExternalInput").ap()
    bout_row_d = nc.dram_tensor("bout_row", [1, D], FP32, kind="ExternalInput").ap()
    cosf_d = nc.dram_tensor("cosf", [P, N], BF16, kind="ExternalInput").ap()
    sinf_d = nc.dram_tensor("sinf", [P, N], BF16, kind="ExternalInput").ap()
    swap_d = nc.dram_tensor("swapm", [P, P], BF16, kind="ExternalInput").ap()
    ones2q_d = nc.dram_tensor("ones2q", [P, 2], BF16, kind="ExternalInput").ap()
    ones2k_d = nc.dram_tensor("ones2k", [P, 2], BF16, kind="ExternalInput").ap()
    vones_d = nc.dram_tensor("vones", [P, NT, H, 1], BF16, kind="ExternalInput").ap()
    out = nc.dram_tensor("out", [N, D], FP32, kind="ExternalOutput").ap()
    rstdq_d = nc.dram_tensor("rstdq_scratch", [H, N], BF16).ap()
    den_d = nc.dram_tensor("den_scratch", [H, N], BF16).ap()
    recd_d = nc.dram_tensor("rec_scratch", [H, N], BF16).ap()

    with tile.TileContext(nc) as tc:
        with tc.tile_pool(name="big", bufs=1) as big, \
             tc.tile_pool(name="tab", bufs=1) as tab, \
             tc.tile_pool(name="wo", bufs=8) as wop:
            xT = big.tile([P, KT, N], BF16)
            qkT = big.tile([P, 2 * KT, N], BF16)     # tile t: heads 2t,2t+1
            vaug = big.tile([P, NT, H, HD + 1], BF16)  # V natural + ones col
            oT_sb = big.tile([P, KT, N], BF16)       # attention out (f-major)
            rstdk = big.tile([P, 2, KT, 2], FP32)    # ring of 2 pgs

            cosf = tab.tile([P, N], BF16)
            sinf = tab.tile([P, N], BF16)
            swapm = tab.tile([P, P], BF16)
            ones2q = tab.tile([P, 2], BF16)
            ones2k = tab.tile([P, 2], BF16)
            bqkv_cols = tab.tile([P, 2 * KT], FP32)
            biasV = tab.tile([P, D], BF16)
            boutB = tab.tile([P, D], FP32)
            eps_t = tab.tile([P, 1], FP32)
            zero_t = tab.tile([P, 1], FP32)
            ln8_t = tab.tile([P, 1], FP32)

            for dst, src in [(cosf, cosf_d), (sinf, sinf_d), (swapm, swap_d),
                             (ones2q, ones2q_d), (ones2k, ones2k_d),
                             (bqkv_cols, bqkv_cols_d)]:
                nc.sync.dma_start(out=dst, in_=src)
            nc.sync.dma_start(out=biasV, in_=bv_row_d.broadcast_to([P, D]))
            nc.sync.dma_start(out=boutB, in_=bout_row_d.broadcast_to([P, D]))
            nc.sync.dma_start(out=vaug[:, :, :, HD:HD + 1], in_=vones_d)
            nc.vector.memset(eps_t, EPS)
            nc.vector.memset(zero_t, 0.0)
            nc.vector.memset(ln8_t, -2.0794415416798357)  # ln(1/8)

            # ---------------- Phase 0: x -> xT (DMA transpose) + V proj ----
            with tc.tile_pool(name="xin", bufs=2) as xin, \
                 tc.tile_pool(name="wv", bufs=8) as wvp, \
                 tc.tile_pool(name="psv", bufs=2, space="PSUM") as psv:
                wvs = []
                for k in range(KT):
                    wv = wvp.tile([P, D], BF16, tag="wv")
                    nc.sync.dma_start(out=wv, in_=wqkv[ds(k * P, P), ds(2 * D, D)])
                    wvs.append(wv)
                for mc in range(NT):
                    xf = xin.tile([P, D], FP32, tag="xf")
                    nc.sync.dma_start(out=xf, in_=x[ds(mc * P, P), :])
                    xb = xin.tile([P, D], BF16, tag="xb")
                    nc.scalar.copy(out=xb, in_=xf)
                    for k in range(KT):
                        nc.sync.dma_start_transpose(
                            xT[:, k, ds(mc * P, P)], xb[:, ds(k * P, P)])
                    # V projection for this n-chunk (stationary xT chunks)
                    pv = psv.tile([P, D], FP32, tag="pv")
                    for k in range(KT):
                        nc.tensor.matmul(
                            pv[:, 0:FB], xT[:, k, ds(mc * P, P)], wvs[k][:, 0:FB],
                            start=(k == 0), stop=(k == KT - 1))
                        nc.tensor.matmul(
                            pv[:, FB:D], xT[:, k, ds(mc * P, P)], wvs[k][:, FB:D],
                            start=(k == 0), stop=(k == KT - 1))
                    nc.vector.tensor_add(
                        out=vaug[:, mc, :, 0:HD],
                        in0=pv.rearrange("p (h d) -> p h d", h=H),
                        in1=biasV.rearrange("p (h d) -> p h d", h=H))

            # ---------------- fused per-head-pair pipeline -----------------
            with tc.tile_pool(name="wqk", bufs=3) as wqkp, \
                 tc.tile_pool(name="sq", bufs=2) as sqp, \
                 tc.tile_pool(name="uc", bufs=4) as ucp, \
                 tc.tile_pool(name="bcp", bufs=2) as bcp, \
                 tc.tile_pool(name="ex", bufs=3) as exp_p, \
                 tc.tile_pool(name="rcp", bufs=2) as rcp, \
                 tc.tile_pool(name="avs", bufs=5) as avsp, \
                 tc.tile_pool(name="dbc", bufs=3) as dbcp, \
                 tc.tile_pool(name="bigp", bufs=3, space="PSUM") as bigp, \
                 tc.tile_pool(name="psav", bufs=2, space="PSUM") as psav:

                def emit_proj(pg, t, ones2, kq):
                    """QKV column-proj + RMSNorm stats for feature tile t."""
                    wcol = wqkp.tile([P, KT, P], BF16, tag="wc")
                    nc.sync.dma_start(
                        out=wcol,
                        in_=wqkv[:, ds(t * P, P)].rearrange(
                            "(ko ki) f -> ki ko f", ki=P))
                    pm = bigp.tile([P, N], FP32, tag="big")
                    for k in range(KT):
                        nc.tensor.matmul(
                            pm[:, 0:FB], wcol[:, k, :], xT[:, k, 0:FB],
                            start=(k == 0), stop=(k == KT - 1))
                        nc.tensor.matmul(
                            pm[:, FB:N], wcol[:, k, :], xT[:, k, FB:N],
                            start=(k == 0), stop=(k == KT - 1))
                    for nb in range(NB):
                        nc.vector.tensor_scalar_add(
                            out=qkT[:, t, ds(nb * FB, FB)], in0=pm[:, ds(nb * FB, FB)],
                            scalar1=bqkv_cols[:, t:t + 1])
                    # RMSNorm stats: sumsq per token, partition-major
                    sq = sqp.tile([P, N], BF16, tag="sq")
                    nc.vector.tensor_mul(out=sq, in0=qkT[:, t, :], in1=qkT[:, t, :])
                    psT = bigp.tile([P, NT, 2], FP32, tag="big")
                    for c in range(NT):
                        nc.tensor.matmul(psT[:, c, :], sq[:, ds(c * P, P)], ones2,
                                         start=True, stop=True)
                    lt = rcp.tile([P, NT * 2], FP32, tag="lt")
                    nc.scalar.activation(
                        out=lt, in_=psT.rearrange("p c h -> p (c h)"),
                        func=AF.Ln, scale=1.0 / HD, bias=eps_t)
                    if kq == "q":
                        rq = rcp.tile([P, 2, NT], BF16, tag="rq")
                        nc.scalar.activation(
                            out=rq.rearrange("p h c -> p c h"), in_=lt,
                            func=AF.Exp, scale=-0.5, bias=zero_t)
                        nc.sync.dma_start(
                            out=rstdq_d[2 * pg:2 * pg + 2, :].rearrange(
                                "h (c p) -> p h c", p=P),
                            in_=rq)
                    else:
                        # 0.125/sigma_k, kept on-chip as exp scale for att
                        nc.scalar.activation(
                            out=rstdk[:, pg % 2, :, :].rearrange("p c h -> p (c h)"),
                            in_=lt, func=AF.Exp, scale=-0.5, bias=ln8_t)

                def emit_rope(pg, t, kq):
                    qs = qkT[:, t, :]
                    u = ucp.tile([P, N], BF16, tag="uc")
                    c = ucp.tile([P, N], BF16, tag="uc")
                    nc.vector.tensor_mul(out=u, in0=qs, in1=sinf)
                    nc.vector.tensor_mul(out=c, in0=qs, in1=cosf)
                    pr = bigp.tile([P, N], FP32, tag="big")
                    nc.tensor.matmul(pr[:, 0:FB], swapm, u[:, 0:FB],
                                     start=True, stop=True)
                    nc.tensor.matmul(pr[:, FB:N], swapm, u[:, FB:N],
                                     start=True, stop=True)
                    if kq == "q":
                        bcq = bcp.tile([P, N], BF16, tag="bc")
                        nc.gpsimd.dma_start(
                            out=bcq[0:HD, :],
                            in_=rstdq_d[2 * pg:2 * pg + 1, :].broadcast_to([HD, N]))
                        nc.gpsimd.dma_start(
                            out=bcq[HD:P, :],
                            in_=rstdq_d[2 * pg + 1:2 * pg + 2, :].broadcast_to([HD, N]))
                        tmp = ucp.tile([P, N], BF16, tag="uc")
                        nc.vector.tensor_add(out=tmp, in0=pr, in1=c)
                        nc.vector.tensor_mul(out=qkT[:, t, :], in0=tmp, in1=bcq)
                    else:
                        nc.vector.tensor_add(out=qkT[:, t, :], in0=pr, in1=c)

                def emit_att_head(pg, h, mcs):
                    """Attention for head 2pg+h over k-chunks mcs."""
                    hh = 2 * pg + h
                    sl = ds(h * HD, HD)
                    for mc in mcs:
                        sp = bigp.tile([P, N], FP32, tag="big")
                        kch = qkT[sl, KT + pg, ds(mc * P, P)]
                        nc.tensor.matmul(sp[:, 0:FB], kch, qkT[sl, pg, 0:FB],
                                         start=True, stop=True)
                        nc.tensor.matmul(sp[:, FB:N], kch, qkT[sl, pg, FB:N],
                                         start=True, stop=True)
                        e = exp_p.tile([P, N], BF16, tag="e")
                        nc.scalar.activation(
                            out=e, in_=sp, func=AF.Exp,
                            scale=rstdk[:, pg % 2, mc, h:h + 1])
                        av = emit_att_head.av
                        nc.tensor.matmul(av[0], vaug[:, mc, hh, :], e[:, 0:FB],
                                         start=(mc == 0), stop=(mc == NT - 1))
                        nc.tensor.matmul(av[1], vaug[:, mc, hh, :], e[:, FB:N],
                                         start=(mc == 0), stop=(mc == NT - 1))

                def att_start(pg, h):
                    av0 = psav.tile([HD + 1, FB], FP32, tag="av")
                    av1 = psav.tile([HD + 1, FB], FP32, tag="av")
                    emit_att_head.av = [av0, av1]

                def att_drain(pg, h):
                    hh = 2 * pg + h
                    for nb in range(NB):
                        avs = avsp.tile([HD + 1, FB], BF16, tag="avs")
                        nc.vector.tensor_copy(out=avs, in_=emit_att_head.av[nb])
                        nc.gpsimd.dma_start(
                            out=den_d[hh:hh + 1, ds(nb * FB, FB)],
                            in_=avs[HD:HD + 1, :])
                        att_drain.avs[(hh, nb)] = avs

                att_drain.avs = {}

                def emit_den(pg):
                    """Batched reciprocal of softmax denominators for pg."""
                    dg = rcp.tile([P, 16], BF16, tag="dg")
                    nc.gpsimd.dma_start(
                        out=dg,
                        in_=den_d[2 * pg:2 * pg + 2, :].rearrange(
                            "h (c q) -> (h c) q", q=16))
                    rec = rcp.tile([P, 16], BF16, tag="rec")
                    with nc.allow_low_precision(reason="bf16 1/den is ample"):
                        nc.vector.reciprocal(out=rec, in_=dg)
                    nc.gpsimd.dma_start(
                        out=recd_d[2 * pg:2 * pg + 2, :].rearrange(
                            "h (c q) -> (h c) q", q=16),
                        in_=rec)
                    for h in range(2):
                        hh = 2 * pg + h
                        for nb in range(NB):
                            dbc = dbcp.tile([HD, FB], BF16, tag="dbc")
                            nc.gpsimd.dma_start(
                                out=dbc,
                                in_=recd_d[hh:hh + 1, ds(nb * FB, FB)].broadcast_to(
                                    [HD, FB]))
                            avs = att_drain.avs.pop((hh, nb))
                            nc.gpsimd.tensor_mul(
                                out=oT_sb[ds(h * HD, HD), pg, ds(nb * FB, FB)],
                                in0=avs[0:HD, :], in1=dbc)

                # schedule: proj/rope of pg interleaved with attention of pg-1
                for pg in range(KT):
                    pa = pg - 1
                    emit_proj(pg, pg, ones2q, "q")
                    if pa >= 0:
                        att_start(pa, 0)
                        emit_att_head(pa, 0, range(0, 4))
                    emit_rope(pg, pg, "q")
                    if pa >= 0:
                        emit_att_head(pa, 0, range(4, 8))
                        att_drain(pa, 0)
                    emit_proj(pg, KT + pg, ones2k, "k")
                    if pa >= 0:
                        att_start(pa, 1)
                        emit_att_head(pa, 1, range(0, 4))
                    emit_rope(pg, KT + pg, "k")
                    if pa >= 0:
                        emit_att_head(pa, 1, range(4, 8))
                        att_drain(pa, 1)
                        emit_den(pa)
                    if pg == KT - 1:
                        for k in range(KT):
                            wo = wop.tile([P, D], BF16, tag="wo")
                            nc.sync.dma_start(out=wo, in_=wout[ds(k * P, P), :])
                            emit_den.wo = getattr(emit_den, "wo", [])
                            emit_den.wo.append(wo)
                pa = KT - 1
                att_start(pa, 0)
                emit_att_head(pa, 0, range(0, 8))
                att_drain(pa, 0)
                att_start(pa, 1)
                emit_att_head(pa, 1, range(0, 8))
                att_drain(pa, 1)
                emit_den(pa)
                wos = emit_den.wo

            # ------------- Phase 4: output projection ------------------
            with tc.tile_pool(name="oout", bufs=2) as ooutp, \
                 tc.tile_pool(name="pso", bufs=2, space="PSUM") as pso:
                for nch in range(NT):
                    po = pso.tile([P, D], FP32, tag="po")
                    for k in range(KT):
                        och = oT_sb[:, k, ds(nch * P, P)]
                        nc.tensor.matmul(po[:, 0:FB], och, wos[k][:, 0:FB],
                                         start=(k == 0), stop=(k == KT - 1))
                        nc.tensor.matmul(po[:, FB:D], och, wos[k][:, FB:D],
                                         start=(k == 0), stop=(k == KT - 1))
                    osb = ooutp.tile([P, D], FP32, tag="osb")
                    nc.vector.tensor_add(out=osb, in0=po, in1=boutB)
                    nc.sync.dma_start(out=out[ds(nch * P, P), :], in_=osb)

    nc.compile()
    return nc


def _host_inputs(Wqkv, bqkv, Wout, bout, q_scale, k_scale):
    import ml_dtypes
    BF = ml_dtypes.bfloat16
    cosF, sinF = _build_tables()

    swapm = np.zeros((P, P), np.float32)
    for k in range(P):
        m = (k & ~63) + ((k & 63) ^ 32)
        swapm[k, m] = 1.0

    # Fold q/k_scale into the Q/K projection columns; the RMSNorm variance of
    # the *unscaled* q is then recovered with a 1/scale^2-weighted reduction.
    qs = q_scale.astype(np.float32)
    ks = k_scale.astype(np.float32)
    W = Wqkv.astype(np.float32).copy()
    b = bqkv.astype(np.float32).copy()
    qcol = np.tile(qs, H)
    kcol = np.tile(ks, H)
    W[:, 0:D] *= qcol[None, :]
    W[:, D:2 * D] *= kcol[None, :]
    b[0:D] *= qcol
    b[D:2 * D] *= kcol

    def wones(sv):
        o = np.zeros((P, 2), np.float32)
        inv2 = 1.0 / (sv * sv)
        o[0:HD, 0] = inv2
        o[HD:P, 1] = inv2
        return o

    bqkv_cols = np.ascontiguousarray(
        b[:2 * D].reshape(2 * KT, P).T).astype(np.float32)

    return {
        "wqkv": W.astype(BF),
        "wout": Wout.astype(np.float32).astype(BF),
        "bqkv_cols": bqkv_cols,
        "bv_row": b[2 * D:].reshape(1, D).astype(BF),
        "bout_row": bout.reshape(1, D).astype(np.float32),
        "cosf": cosF.astype(BF), "sinf": sinF.astype(BF),
        "swapm": swapm.astype(BF),
        "ones2q": wones(qs).astype(BF), "ones2k": wones(ks).astype(BF),
        "vones": np.ones((P, NT, H, 1), np.float32).astype(BF),
    }


def _get_built():
    global _BUILT
    if _BUILT is None:
        _BUILT = _build_program()
    return _BUILT


def kernel(x, Wqkv, bqkv, Wout, bout, q_scale, k_scale, _trace=False):
    from concourse.bass_utils import run_bass_kernel_spmd

    x = np.asarray(x, dtype=np.float32)
    shared = _host_inputs(np.asarray(Wqkv, np.float32), np.asarray(bqkv, np.float32),
                          np.asarray(Wout, np.float32), np.asarray(bout, np.float32),
                          np.asarray(q_scale, np.float32), np.asarray(k_scale, np.float32))
    in_maps = [dict(shared, x=np.ascontiguousarray(x[c])) for c in range(B)]
    nc = _get_built()
    res = run_bass_kernel_spmd(nc, in_maps, core_ids=list(range(B)), trace=_trace)
    out = np.stack([res.results[c]["out"] for c in range(B)], axis=0)
    kernel.last_exec_time_ns = res.exec_time_ns
    kernel.last_results = res
    return out


# revision 18
# speedup vs baseline: 1.7814x; 1.1574x over previous
"""Fused multi-head attention block (QKV proj + RMSNorm + 2D RoPE + softmax
attention + out proj) for Trainium2, data-parallel over batch on 8 NeuronCores.

v2 layout strategy per core (one batch element, N=1024, D=1024, H=16, hd=64):
  - All PE operands are bf16 (weights host-cast; x cast on-chip); PSUM
    accumulates fp32. bf16 enables fast-weight-load and 2x DVE modes.
  - x is transposed to xT [D, N] by the DMA XBAR (dma_start_transpose), not
    the PE.
  - Q,K are produced transposed ("qkT" [feat, n]); V in natural [n, feat]
    layout augmented with a ones column so the softmax denominator falls out
    of the AV accumulation.
  - Matmuls write [128, 1024] two-bank PSUM groups so one weight load
    streams 1024 columns (halves LDWEIGHTS count).
  - RMSNorm sumsq is computed per 128-token chunk into a [128, 8, 2]
    partition-major PSUM tile -> two small ACTs (ln, exp) per feature tile.
    Q-side rstd is applied via 0-stride broadcast DMA after RoPE; K-side
    rstd (with hd^-0.5 folded in) becomes the per-partition scale of the
    softmax exp ACT.
  - RoPE rotate-half: two DVE table-multiplies, one PE swap-matrix matmul,
    and a DVE add (no identity matmul).
  - Softmax denominator reciprocals are batched into one [128, 16] DVE op
    per head pair via a DRAM-rearrange round trip; the 1/den multiply runs
    on the (otherwise idle) GpSimd engine.
  - Attention output oT stays in SBUF for the final projection.
Softmax skips max-subtraction: after RMSNorm ||q||<=8, ||k||<=8 so logits
are within [-8, 8], safely inside exp range.
"""

import sys

sys.path.insert(0, "/opt/trn_rl_repo")

import numpy as np

_BUILT = None

B, N, D = 8, 1024, 1024
H, HD = 16, 64
P = 128
NB = 2          # free-dim blocks of 512 over n
FB = 512
KT = D // P     # 8 contraction chunks
NT = N // P     # 8 n-chunks
THETA = 10000.0
EPS = 1e-6


def _rope_tables():
    side = int(np.sqrt(N))
    dq = HD // 4
    inv_freq = 1.0 / (THETA ** (np.arange(dq, dtype=np.float32) / dq))
    ang = np.arange(side, dtype=np.float32)[:, None] * inv_freq[None, :]
    row = np.broadcast_to(ang[:, None, :], (side, side, dq)).reshape(N, dq)
    col = np.broadcast_to(ang[None, :, :], (side, side, dq)).reshape(N, dq)
    angles = np.concatenate([row, col], axis=-1)  # [N, 32]
    return np.cos(angles), np.sin(angles)


def _build_tables():
    """cosF/sinF' [128, N] for a 2-head tile (rows: head-even dims 0..63,
    then head-odd dims 0..63). sinF'[i] carries the rotate-half sign."""
    cos, sin = _rope_tables()  # [N, 32] each
    cosF = np.empty((P, N), np.float32)
    sinF = np.empty((P, N), np.float32)
    for i in range(P):
        d = i % HD
        a = d % 32
        cosF[i] = cos[:, a]
        sinF[i] = sin[:, a] * (1.0 if d < 32 else -1.0)
    return cosF, sinF


def _build_program():
    import concourse.bass as bass
    import concourse.mybir as mybir
    import concourse.tile as tile
    from concourse import bacc
    from concourse.bass import ds

    # Keep every ACT function this kernel uses (ln, exp, copy) in a single
    # table set so the table-load pass emits exactly one load.
    if not getattr(bacc, "_act_tables_patched", False):
        _orig_get_tables = bacc.get_activation_tables

        def _only_lnexp(arch):
            import concourse.mybir as _mb
            tabs = _orig_get_tables(arch)
            if "natural_log_exp_and_others" not in tabs:
                return tabs
            steer = set()
            for fname in ("Exp", "Ln", "Copy", "Identity", "Square"):
                steer.add(getattr(_mb.ActivationFunctionType, fname))
            out = {}
            for name, funcs in tabs.items():
                if name == "natural_log_exp_and_others":
                    out[name] = funcs
                else:
                    out[name] = funcs - steer
            return out

        bacc.get_activation_tables = _only_lnexp
        bacc._act_tables_patched = True

    BF16 = mybir.dt.bfloat16
    FP32 = mybir.dt.float32
    AF = mybir.ActivationFunctionType

    nc = bacc.Bacc("TRN2", target_bir_lowering=False, debug=False, num_devices=8)

    x = nc.dram_tensor("x", [N, D], FP32, kind="ExternalInput").ap()
    wqkv = nc.dram_tensor("wqkv", [D, 3 * D], BF16, kind="ExternalInput").ap()
    wout = nc.dram_tensor("wout", [D, D], BF16, kind="ExternalInput").ap()
    bqkv_cols_d = nc.dram_tensor("bqkv_cols", [P, 2 * KT], FP32, kind="ExternalInput").ap()
    bv_row_d = nc.dram_tensor("bv_row", [1, D], BF16, kind="ExternalInput").ap()
    bout_row_d = nc.dram_tensor("bout_row", [1, D], FP32, kind="ExternalInput").ap()
    cosf_d = nc.dram_tensor("cosf", [P, N], BF16, kind="ExternalInput").ap()
    sinf_d = nc.dram_tensor("sinf", [P, N], BF16, kind="ExternalInput").ap()
    swap_d = nc.dram_tensor("swapm", [P, P], BF16, kind="ExternalInput").ap()
    ones2q_d = nc.dram_tensor("ones2q", [P, 2], BF16, kind="ExternalInput").ap()
    ones2k_d = nc.dram_tensor("ones2k", [P, 2], BF16, kind="ExternalInput").ap()
    ident_d = nc.dram_tensor("ident", [P, P], BF16, kind="ExternalInput").ap()
    vones_d = nc.dram_tensor("vones", [P, NT, H, 1], BF16, kind="ExternalInput").ap()
    out = nc.dram_tensor("out", [N, D], FP32, kind="ExternalOutput").ap()
    rstdq_d = nc.dram_tensor("rstdq_scratch", [H, N], BF16).ap()
    den_d = nc.dram_tensor("den_scratch", [H, N], BF16).ap()
    recd_d = nc.dram_tensor("rec_scratch", [H, N], BF16).ap()

    with tile.TileContext(nc) as tc:
        with tc.tile_pool(name="big", bufs=1) as big, \
             tc.tile_pool(name="tab", bufs=1) as tab, \
             tc.tile_pool(name="wo", bufs=8) as wop:
            xT = big.tile([P, KT, N], BF16)
            qkT = big.tile([P, 2 * KT, N], BF16)     # tile t: heads 2t,2t+1
            vaug = big.tile([P, NT, H, HD + 1], BF16)  # V natural + ones col
            oT_sb = big.tile([P, KT, N], BF16)       # attention out (f-major)
            rstdk = big.tile([P, 2, KT, 2], FP32)    # ring of 2 pgs

            cosf = tab.tile([P, N], BF16)
            sinf = tab.tile([P, N], BF16)
            swapm = tab.tile([P, P], BF16)
            ident = tab.tile([P, P], BF16)
            ones2q = tab.tile([P, 2], BF16)
            ones2k = tab.tile([P, 2], BF16)
            bqkv_cols = tab.tile([P, 2 * KT], FP32)
            biasV = tab.tile([P, D], BF16)
            boutB = tab.tile([P, D], FP32)
            eps_t = tab.tile([P, 1], FP32)
            zero_t = tab.tile([P, 1], FP32)
            ln8_t = tab.tile([P, 1], FP32)

            for dst, src in [(cosf, cosf_d), (sinf, sinf_d), (swapm, swap_d),
                             (ident, ident_d),
                             (ones2q, ones2q_d), (ones2k, ones2k_d),
                             (bqkv_cols, bqkv_cols_d)]:
                nc.sync.dma_start(out=dst, in_=src)
            nc.sync.dma_start(out=biasV, in_=bv_row_d.broadcast_to([P, D]))
            nc.sync.dma_start(out=boutB, in_=bout_row_d.broadcast_to([P, D]))
            nc.sync.dma_start(out=vaug[:, :, :, HD:HD + 1], in_=vones_d)
            nc.vector.memset(eps_t, EPS)
            nc.vector.memset(zero_t, 0.0)
            nc.vector.memset(ln8_t, -2.0794415416798357)  # ln(1/8)

            # ------- Phase 0: x -> xT (PE transpose, bf16) + V proj --------
            with tc.tile_pool(name="xin", bufs=2) as xin, \
                 tc.tile_pool(name="wv", bufs=8) as wvp, \
                 tc.tile_pool(name="pst", bufs=3, space="PSUM") as pst, \
                 tc.tile_pool(name="psv", bufs=2, space="PSUM") as psv:
                wvs = []
                for k in range(KT):
                    wv = wvp.tile([P, D], BF16, tag="wv")
                    nc.sync.dma_start(out=wv, in_=wqkv[ds(k * P, P), ds(2 * D, D)])
                    wvs.append(wv)
                I32 = mybir.dt.int32
                for mc in range(NT):
                    xf = xin.tile([P, D], FP32, tag="xf")
                    nc.sync.dma_start(out=xf, in_=x[ds(mc * P, P), :])
                    xb = xin.tile([P, D], BF16, tag="xb")
                    nc.scalar.copy(out=xb, in_=xf)
                    for half in range(2):
                        pxt = pst.tile([P, 4, P], BF16, tag="pxt")
                        for j in range(4):
                            nc.tensor.transpose(
                                pxt[:, j, :], xb[:, ds((half * 4 + j) * P, P)],
                                ident)
                        nc.vector.tensor_copy(
                            out=xT[:, ds(half * 4, 4), ds(mc * P, P)].bitcast(I32),
                            in_=pxt.bitcast(I32))
                    # V projection for this n-chunk (stationary xT chunks)
                    pv = psv.tile([P, D], FP32, tag="pv")
                    for k in range(KT):
                        nc.tensor.matmul(
                            pv[:, 0:FB], xT[:, k, ds(mc * P, P)], wvs[k][:, 0:FB],
                            start=(k == 0), stop=(k == KT - 1))
                        nc.tensor.matmul(
                            pv[:, FB:D], xT[:, k, ds(mc * P, P)], wvs[k][:, FB:D],
                            start=(k == 0), stop=(k == KT - 1))
                    nc.vector.tensor_add(
                        out=vaug[:, mc, :, 0:HD],
                        in0=pv.rearrange("p (h d) -> p h d", h=H),
                        in1=biasV.rearrange("p (h d) -> p h d", h=H))

            # ---------------- fused per-head-pair pipeline -----------------
            with tc.tile_pool(name="wqk", bufs=3) as wqkp, \
                 tc.tile_pool(name="sq", bufs=2) as sqp, \
                 tc.tile_pool(name="uc", bufs=6) as ucp, \
                 tc.tile_pool(name="bcp", bufs=2) as bcp, \
                 tc.tile_pool(name="ex", bufs=3) as exp_p, \
                 tc.tile_pool(name="rcp", bufs=2) as rcp, \
                 tc.tile_pool(name="avs", bufs=5) as avsp, \
                 tc.tile_pool(name="dbc", bufs=3) as dbcp, \
                 tc.tile_pool(name="bigp", bufs=3, space="PSUM") as bigp, \
                 tc.tile_pool(name="psav", bufs=2, space="PSUM") as psav:

                def emit_proj(pg, t, ones2, kq):
                    """QKV column-proj + RMSNorm stats for feature tile t."""
                    wcol = wqkp.tile([P, KT, P], BF16, tag="wc")
                    nc.sync.dma_start(
                        out=wcol,
                        in_=wqkv[:, ds(t * P, P)].rearrange(
                            "(ko ki) f -> ki ko f", ki=P))
                    pm = bigp.tile([P, N], FP32, tag="big")
                    for k in range(KT):
                        nc.tensor.matmul(
                            pm[:, 0:FB], wcol[:, k, :], xT[:, k, 0:FB],
                            start=(k == 0), stop=(k == KT - 1))
                        nc.tensor.matmul(
                            pm[:, FB:N], wcol[:, k, :], xT[:, k, FB:N],
                            start=(k == 0), stop=(k == KT - 1))
                    for nb in range(NB):
                        nc.vector.tensor_scalar_add(
                            out=qkT[:, t, ds(nb * FB, FB)], in0=pm[:, ds(nb * FB, FB)],
                            scalar1=bqkv_cols[:, t:t + 1])
                    # RMSNorm stats: sumsq per token, partition-major
                    sq = sqp.tile([P, N], BF16, tag="sq")
                    nc.vector.tensor_mul(out=sq, in0=qkT[:, t, :], in1=qkT[:, t, :])
                    psT = bigp.tile([P, NT, 2], FP32, tag="big")
                    for c in range(NT):
                        nc.tensor.matmul(psT[:, c, :], sq[:, ds(c * P, P)], ones2,
                                         start=True, stop=True)
                    lt = rcp.tile([P, NT * 2], FP32, tag="lt")
                    nc.scalar.activation(
                        out=lt, in_=psT.rearrange("p c h -> p (c h)"),
                        func=AF.Ln, scale=1.0 / HD, bias=eps_t)
                    if kq == "q":
                        rq = rcp.tile([P, 2, NT], BF16, tag="rq")
                        nc.scalar.activation(
                            out=rq.rearrange("p h c -> p c h"), in_=lt,
                            func=AF.Exp, scale=-0.5, bias=zero_t)
                        nc.sync.dma_start(
                            out=rstdq_d[2 * pg:2 * pg + 2, :].rearrange(
                                "h (c p) -> p h c", p=P),
                            in_=rq)
                    else:
                        # 0.125/sigma_k, kept on-chip as exp scale for att
                        nc.scalar.activation(
                            out=rstdk[:, pg % 2, :, :].rearrange("p c h -> p (c h)"),
                            in_=lt, func=AF.Exp, scale=-0.5, bias=ln8_t)

                def emit_rope(pg, t, kq):
                    qs = qkT[:, t, :]
                    u = ucp.tile([P, N], BF16, tag="uc")
                    c = ucp.tile([P, N], BF16, tag="uc")
                    nc.vector.tensor_mul(out=u, in0=qs, in1=sinf)
                    nc.vector.tensor_mul(out=c, in0=qs, in1=cosf)
                    pr = bigp.tile([P, N], FP32, tag="big")
                    nc.tensor.matmul(pr[:, 0:FB], swapm, u[:, 0:FB],
                                     start=True, stop=True)
                    nc.tensor.matmul(pr[:, FB:N], swapm, u[:, FB:N],
                                     start=True, stop=True)
                    if kq == "q":
                        # stash rotated-but-unscaled q; the rstd_q multiply is
                        # deferred one iteration so the DRAM round trip +
                        # broadcast DMA never stall the vector FIFO.
                        bcq = bcp.tile([P, N], BF16, tag="bc")
                        nc.sync.dma_start(
                            out=bcq[0:HD, :],
                            in_=rstdq_d[2 * pg:2 * pg + 1, :].broadcast_to([HD, N]))
                        nc.sync.dma_start(
                            out=bcq[HD:P, :],
                            in_=rstdq_d[2 * pg + 1:2 * pg + 2, :].broadcast_to([HD, N]))
                        tmp = ucp.tile([P, N], BF16, tag="tmpq")
                        nc.vector.tensor_add(out=tmp, in0=pr, in1=c)
                        emit_rope.pending = (pg, tmp, bcq)
                    else:
                        nc.vector.tensor_add(out=qkT[:, t, :], in0=pr, in1=c)

                def apply_q(pg):
                    pg_, tmp, bcq = emit_rope.pending
                    assert pg_ == pg
                    nc.vector.tensor_mul(out=qkT[:, pg, :], in0=tmp, in1=bcq)

                def emit_att_head(pg, h, mcs):
                    """Attention for head 2pg+h over k-chunks mcs."""
                    hh = 2 * pg + h
                    sl = ds(h * HD, HD)
                    for mc in mcs:
                        sp = bigp.tile([P, N], FP32, tag="big")
                        kch = qkT[sl, KT + pg, ds(mc * P, P)]
                        nc.tensor.matmul(sp[:, 0:FB], kch, qkT[sl, pg, 0:FB],
                                         start=True, stop=True)
                        nc.tensor.matmul(sp[:, FB:N], kch, qkT[sl, pg, FB:N],
                                         start=True, stop=True)
                        e = exp_p.tile([P, N], BF16, tag="e")
                        nc.scalar.activation(
                            out=e, in_=sp, func=AF.Exp,
                            scale=rstdk[:, pg % 2, mc, h:h + 1])
                        av = emit_att_head.av
                        nc.tensor.matmul(av[0], vaug[:, mc, hh, :], e[:, 0:FB],
                                         start=(mc == 0), stop=(mc == NT - 1))
                        nc.tensor.matmul(av[1], vaug[:, mc, hh, :], e[:, FB:N],
                                         start=(mc == 0), stop=(mc == NT - 1))

                def att_start(pg, h):
                    av0 = psav.tile([HD + 1, FB], FP32, tag="av")
                    av1 = psav.tile([HD + 1, FB], FP32, tag="av")
                    emit_att_head.av = [av0, av1]

                def att_drain(pg, h):
                    hh = 2 * pg + h
                    for nb in range(NB):
                        avs = avsp.tile([HD + 1, FB], BF16, tag="avs")
                        nc.vector.tensor_copy(out=avs, in_=emit_att_head.av[nb])
                        nc.gpsimd.dma_start(
                            out=den_d[hh:hh + 1, ds(nb * FB, FB)],
                            in_=avs[HD:HD + 1, :])
                        att_drain.avs[(hh, nb)] = avs

                att_drain.avs = {}

                def emit_den(pg):
                    """Batched reciprocal of softmax denominators for pg."""
                    dg = rcp.tile([P, 16], BF16, tag="dg")
                    nc.gpsimd.dma_start(
                        out=dg,
                        in_=den_d[2 * pg:2 * pg + 2, :].rearrange(
                            "h (c q) -> (h c) q", q=16))
                    rec = rcp.tile([P, 16], BF16, tag="rec")
                    with nc.allow_low_precision(reason="bf16 1/den is ample"):
                        nc.vector.reciprocal(out=rec, in_=dg)
                    nc.gpsimd.dma_start(
                        out=recd_d[2 * pg:2 * pg + 2, :].rearrange(
                            "h (c q) -> (h c) q", q=16),
                        in_=rec)
                    for h in range(2):
                        hh = 2 * pg + h
                        for nb in range(NB):
                            dbc = dbcp.tile([HD, FB], BF16, tag="dbc")
                            nc.gpsimd.dma_start(
                                out=dbc,
                                in_=recd_d[hh:hh + 1, ds(nb * FB, FB)].broadcast_to(
                                    [HD, FB]))
                            avs = att_drain.avs.pop((hh, nb))
                            nc.gpsimd.tensor_mul(
                                out=oT_sb[ds(h * HD, HD), pg, ds(nb * FB, FB)],
                                in0=avs[0:HD, :], in1=dbc)

                # schedule: proj/rope of pg interleaved with attention of
                # pg-1; the q-side rstd multiply of pg-1 lands at the top of
                # iteration pg (one full iteration of DMA slack).
                for pg in range(KT):
                    pa = pg - 1
                    if pa >= 0:
                        apply_q(pa)
                    emit_proj(pg, pg, ones2q, "q")
                    if pa >= 0:
                        att_start(pa, 0)
                        emit_att_head(pa, 0, range(0, 4))
                    emit_rope(pg, pg, "q")
                    if pa >= 0:
                        emit_att_head(pa, 0, range(4, 8))
                        att_drain(pa, 0)
                    emit_proj(pg, KT + pg, ones2k, "k")
                    if pa >= 0:
                        att_start(pa, 1)
                        emit_att_head(pa, 1, range(0, 4))
                    emit_rope(pg, KT + pg, "k")
                    if pa >= 0:
                        emit_att_head(pa, 1, range(4, 8))
                        att_drain(pa, 1)
                        emit_den(pa)
                    if pg == KT - 1:
                        for k in range(KT):
                            wo = wop.tile([P, D], BF16, tag="wo")
                            nc.sync.dma_start(out=wo, in_=wout[ds(k * P, P), :])
                            emit_den.wo = getattr(emit_den, "wo", [])
                            emit_den.wo.append(wo)
                pa = KT - 1
                apply_q(pa)
                att_start(pa, 0)
                emit_att_head(pa, 0, range(0, 8))
                att_drain(pa, 0)
                att_start(pa, 1)
                emit_att_head(pa, 1, range(0, 8))
                att_drain(pa, 1)
                emit_den(pa)
                wos = emit_den.wo

            # ------------- Phase 4: output projection ------------------
            with tc.tile_pool(name="oout", bufs=2) as ooutp, \
                 tc.tile_pool(name="pso", bufs=2, space="PSUM") as pso:
                for nch in range(NT):
                    po = pso.tile([P, D], FP32, tag="po")
                    for k in range(KT):
                        och = oT_sb[:, k, ds(nch * P, P)]
                        nc.tensor.matmul(po[:, 0:FB], och, wos[k][:, 0:FB],
                                         start=(k == 0), stop=(k == KT - 1))
                        nc.tensor.matmul(po[:, FB:D], och, wos[k][:, FB:D],
                                         start=(k == 0), stop=(k == KT - 1))
                    osb = ooutp.tile([P, D], FP32, tag="osb")
                    nc.vector.tensor_add(out=osb, in0=po, in1=boutB)
                    nc.sync.dma_start(out=out[ds(nch * P, P), :], in_=osb)

    nc.compile()
    return nc


def _host_inputs(Wqkv, bqkv, Wout, bout, q_scale, k_scale):
    import ml_dtypes
    BF = ml_dtypes.bfloat16
    cosF, sinF = _build_tables()

    swapm = np.zeros((P, P), np.float32)
    for k in range(P):
        m = (k & ~63) + ((k & 63) ^ 32)
        swapm[k, m] = 1.0

    # Fold q/k_scale into the Q/K projection columns; the RMSNorm variance of
    # the *unscaled* q is then recovered with a 1/scale^2-weighted reduction.
    qs = q_scale.astype(np.float32)
    ks = k_scale.astype(np.float32)
    W = Wqkv.astype(np.float32).copy()
    b = bqkv.astype(np.float32).copy()
    qcol = np.tile(qs, H)
    kcol = np.tile(ks, H)
    W[:, 0:D] *= qcol[None, :]
    W[:, D:2 * D] *= kcol[None, :]
    b[0:D] *= qcol
    b[D:2 * D] *= kcol

    def wones(sv):
        o = np.zeros((P, 2), np.float32)
        inv2 = 1.0 / (sv * sv)
        o[0:HD, 0] = inv2
        o[HD:P, 1] = inv2
        return o

    bqkv_cols = np.ascontiguousarray(
        b[:2 * D].reshape(2 * KT, P).T).astype(np.float32)

    return {
        "wqkv": W.astype(BF),
        "wout": Wout.astype(np.float32).astype(BF),
        "bqkv_cols": bqkv_cols,
        "bv_row": b[2 * D:].reshape(1, D).astype(BF),
        "bout_row": bout.reshape(1, D).astype(np.float32),
        "cosf": cosF.astype(BF), "sinf": sinF.astype(BF),
        "swapm": swapm.astype(BF),
        "ones2q": wones(qs).astype(BF), "ones2k": wones(ks).astype(BF),
        "ident": np.eye(P, dtype=np.float32).astype(BF),
        "vones": np.ones((P, NT, H, 1), np.float32).astype(BF),
    }


def _get_built():
    global _BUILT
    if _BUILT is None:
        _BUILT = _build_program()
    return _BUILT


def kernel(x, Wqkv, bqkv, Wout, bout, q_scale, k_scale, _trace=False):
    from concourse.bass_utils import run_bass_kernel_spmd

    x = np.asarray(x, dtype=np.float32)
    shared = _host_inputs(np.asarray(Wqkv, np.float32), np.asarray(bqkv, np.float32),
                          np.asarray(Wout, np.float32), np.asarray(bout, np.float32),
                          np.asarray(q_scale, np.float32), np.asarray(k_scale, np.float32))
    in_maps = [dict(shared, x=np.ascontiguousarray(x[c])) for c in range(B)]
    nc = _get_built()
    res = run_bass_kernel_spmd(nc, in_maps, core_ids=list(range(B)), trace=_trace)
    out = np.stack([res.results[c]["out"] for c in range(B)], axis=0)
    kernel.last_exec_time_ns = res.exec_time_ns
    kernel.last_results = res
    return out
